# revision 1
# baseline (speedup 1.0000x reference)
"""Triangular pairwise channel product on 8 Trainium2 NeuronCores.

out[b,h,w,k] = x[b,h,w,i_k] * x[b,h,w,j_k]  for the C*(C-1)/2 pairs
(i<j) in row-major (np.triu_indices) order.

Sharding: pure data parallel over batch — core c takes x[2c:2c+2].
Per core the 2*64*64 = 8192 spatial positions map to 128 SBUF
partitions (b_loc*64+h) x 64 groups (w).  For each group-block of G
positions, block i of the output (pairs (i, i+1..63)) is one fp32
tensor_tensor multiply whose first operand is x[:, :, i] broadcast via
a step-0 access pattern — 63 DVE ops per iteration cover all 2016
output channels with per-partition-contiguous stores back to HBM.
Iteration sizes taper (G_ITERS) so the final HBM drain after the last
compute op stays small; ~204 us/kernel vs a ~190 us write roofline.
"""

import numpy as np

import concourse.bacc as bacc
import concourse.bass as bass
import concourse.mybir as mybir
import concourse.tile as tile
from concourse.bass_utils import run_bass_kernel_spmd

B, H, W, C = 16, 64, 64, 64
K = C * (C - 1) // 2  # 2016
N_CORES = 8
BP = B // N_CORES  # batch rows per core
P = BP * H         # 128 SBUF partitions
G_TOTAL = W        # position groups per partition
# Iteration group sizes: G=8 steady state (compute ~= DMA per group), a
# two-step tail so the post-compute drain balances against tail compute.
# Measured-best among: all-8, front/back tapers, G=10 steady, descending
# sizes (worse slot-release slips under bufs=2), and bufs=3 (gapless DVE
# starves the output DMA of SBUF bank bandwidth, ~360 vs ~429 GB/s).
G_ITERS = [8, 8, 8, 8, 8, 8, 8, 5, 3]
assert sum(G_ITERS) == W
G0 = G_ITERS[0]
FP = mybir.dt.float32

_row = [0]
for _i in range(C):
    _row.append(_row[-1] + C - 1 - _i)

_nc_cache = None


def build_bass() -> bass.Bass:
    # Bacc (not plain Bass): its compile() pipeline runs
    # generate_event_semaphores, which splits multi-wait instructions to
    # satisfy the TRN2 1-wait-per-instruction codegen limit.
    nc = bacc.Bacc(
        "TRN2",
        target_bir_lowering=False,
        debug=False,
        num_devices=N_CORES,
    )
    x = nc.dram_tensor("x", [P, G_TOTAL, C], FP, kind="ExternalInput")
    y = nc.dram_tensor("y", [P, G_TOTAL, K], FP, kind="ExternalOutput")

    with tile.TileContext(nc) as tc:
        with (
            tc.tile_pool(name="xin", bufs=1) as xpool,
            tc.tile_pool(name="out", bufs=2) as opool,
        ):
            # Preload the input in two pieces: iteration 0's chunk on the
            # SP ring (out0 isn't due on it until ~20 us), the rest on
            # the ACT ring so neither load queues behind output stores.
            xt0 = xpool.tile([P, G0, C], FP, tag="x0")
            nc.sync.dma_start(out=xt0[:], in_=x[:, 0:G0, :])
            xtr = xpool.tile([P, G_TOTAL - G0, C], FP, tag="xr")
            nc.scalar.dma_start(out=xtr[:], in_=x[:, G0:, :])

            g_off = 0
            for it, Gi in enumerate(G_ITERS):
                if it == 0:
                    xg = xt0[:, :, :]
                else:
                    xg = xtr[:, g_off - G0 : g_off - G0 + Gi, :]

                # All output stores ride the SP ring with full 2016-channel
                # rows (contiguous per-partition DRAM runs).
                ot = opool.tile([P, Gi, K], FP, tag="ot")
                for i in range(C - 1):
                    w = C - 1 - i
                    a = xg[:, :, i : i + 1].broadcast_to([P, Gi, w])
                    b = xg[:, :, i + 1 : C]
                    nc.vector.tensor_mul(ot[:, :, _row[i] : _row[i] + w], a, b)

                nc.sync.dma_start(out=y[:, g_off : g_off + Gi, :], in_=ot[:])
                g_off += Gi

    nc.finalize()
    return nc


def make_in_maps(x: np.ndarray) -> list[dict[str, np.ndarray]]:
    x = np.ascontiguousarray(x, dtype=np.float32)
    return [
        {"x": x[c * BP : (c + 1) * BP].reshape(P, G_TOTAL, C)} for c in range(N_CORES)
    ]


def kernel(**inputs: np.ndarray) -> np.ndarray:
    global _nc_cache
    if _nc_cache is None:
        _nc_cache = build_bass()
    res = run_bass_kernel_spmd(
        _nc_cache, make_in_maps(inputs["inputs"]), list(range(N_CORES))
    ).results
    return np.concatenate(
        [res[c]["y"].reshape(BP, H, W, K) for c in range(N_CORES)], axis=0
    )

